# revision 1
# baseline (speedup 1.0000x reference)
"""ConflateLoss Trainium2 kernel.

loss = -sum_b log_softmax(10 * cos_sim(anchor_b, match[cand_idx_b]))[0] / ln(10)
with B=16384, D=128, 50 candidates per anchor (slot 0 = positive b, 1..49 = neg_idx).

Strategy (8 NeuronCores, data-parallel over B):
  Phase 0 (each core): normalize match_embedding rows to unit L2 norm, cast to
    bf16, stage to a private DRAM buffer `mhat`. Normalize this core's 2048
    anchors by 10/||a|| (gamma folded) into SBUF bf16.
  Phase 1: per 128-anchor block, one dma_gather pulls the 6400 candidate rows
    (bf16, 256B each) from mhat into SBUF laid out [anchor_part, slot, d].
    DVE: prod = cand * anchor (broadcast over slots), two-stage add-reduce over
    d -> logits f32 [128, 50]. ACT: Exp with accum_out -> softmax denominator
    in one instruction, Ln, then (ln(denom) - logit0) per anchor.
  Output: [128, 16] per-core partial losses; host sums and divides by ln(10).
"""

import math

import numpy as np

import concourse.bacc as bacc
import concourse.bass as bass
import concourse.tile as tile
from concourse import mybir
from concourse.bass_utils import run_bass_kernel_spmd

B = 16384
D = 128
N_NEG = 49
N_CAND = 50
N_CORES = 8
B_SHARD = B // N_CORES  # 2048 anchors per core
N_BLK = B_SHARD // 128  # 16 blocks of 128 anchors
N_IDX = 128 * N_CAND  # 6400 gathered rows per block
IDX_COLS = N_IDX // 16  # 400 (dma_gather index wrap: token i at [i%16, i//16])
M_GROUPS = 16  # match rows processed in 16 groups of 1024
G_ROWS = B // M_GROUPS // 128  # 8 rows per partition per group

F32 = mybir.dt.float32
BF16 = mybir.dt.bfloat16
AF = mybir.ActivationFunctionType


def _bcast_mid(ap: bass.AP, count: int) -> bass.AP:
    """[128, D] AP -> [128, count, D] with a stride-0 middle dim."""
    assert len(ap.ap) == 2
    return bass.AP(
        tensor=ap.tensor,
        offset=ap.offset,
        ap=[list(ap.ap[0]), [0, count], list(ap.ap[1])],
    )


def build_bass() -> bacc.Bacc:
    nc = bacc.Bacc("TRN2", debug=False, num_devices=N_CORES)

    match_in = nc.dram_tensor("match", [B, D], F32, kind="ExternalInput")
    anchors_in = nc.dram_tensor("anchors", [B_SHARD, D], F32, kind="ExternalInput")
    idx_in = nc.dram_tensor("idx", [128, N_BLK, IDX_COLS], mybir.dt.int16,
                            kind="ExternalInput")
    out = nc.dram_tensor("out", [128, N_BLK], F32, kind="ExternalOutput")

    with tile.TileContext(nc) as tc:
        with (
            tc.tile_pool(name="dram", bufs=1, space="DRAM") as dram_pool,
            tc.tile_pool(name="mload", bufs=2) as mload,
            tc.tile_pool(name="msq", bufs=2) as msq,
            tc.tile_pool(name="mnorm", bufs=3) as mnorm,
            tc.tile_pool(name="mhatsb", bufs=2) as mhatsb,
            tc.tile_pool(name="single", bufs=1) as single,
            tc.tile_pool(name="cand", bufs=3) as candp,
            tc.tile_pool(name="prod", bufs=2) as prodp,
            tc.tile_pool(name="small", bufs=4) as small,
        ):
            mhat = dram_pool.tile([B, D], BF16)
            # row r of mhat/match handled by partition (r % 1024) // 8 of
            # group r // 1024 -> per-partition-contiguous DMA both ways.
            match_r = match_in.ap().rearrange(
                "(g p t) d -> g p t d", p=128, t=G_ROWS)
            mhat_r = mhat[:].rearrange("(g p t) d -> g p t d", p=128, t=G_ROWS)

            # ---- Phase 0a: normalize match rows -> mhat (bf16, unit norm)
            for g in range(M_GROUPS):
                mf = mload.tile([128, G_ROWS, D], F32)
                nc.sync.dma_start(out=mf, in_=match_r[g])
                sq = msq.tile([128, G_ROWS, D], F32)
                nc.scalar.activation(out=sq, in_=mf, func=AF.Square)
                nsq = mnorm.tile([128, G_ROWS], F32)
                nc.vector.tensor_reduce(
                    out=nsq, in_=sq, axis=mybir.AxisListType.X,
                    op=mybir.AluOpType.add)
                rin = mnorm.tile([128, G_ROWS], F32)
                nc.vector.reciprocal(out=rin, in_=nsq)
                inv = mnorm.tile([128, G_ROWS], F32)
                nc.scalar.activation(out=inv, in_=rin, func=AF.Sqrt)
                mh = mhatsb.tile([128, G_ROWS, D], BF16)
                for t in range(G_ROWS):
                    nc.scalar.mul(out=mh[:, t, :], in_=mf[:, t, :],
                                  mul=inv[:, t:t + 1])
                nc.sync.dma_start(out=mhat_r[g], in_=mh)

            # ---- Phase 0b: anchors -> bf16, scaled by 10/||a||
            anch_r = anchors_in.ap().rearrange("(k p) d -> p k d", p=128)
            af = single.tile([128, N_BLK, D], F32)
            nc.sync.dma_start(out=af, in_=anch_r)
            asq = msq.tile([128, N_BLK, D], F32, tag="msq")
            nc.scalar.activation(out=asq, in_=af, func=AF.Square)
            nsqa = single.tile([128, N_BLK], F32)
            nc.vector.tensor_reduce(
                out=nsqa, in_=asq, axis=mybir.AxisListType.X,
                op=mybir.AluOpType.add)
            ra = single.tile([128, N_BLK], F32)
            nc.vector.reciprocal(out=ra, in_=nsqa)
            sca = single.tile([128, N_BLK], F32)
            # sqrt(100/nsq) = 10/||a||  (gamma=10 folded into the anchor)
            nc.scalar.activation(out=sca, in_=ra, func=AF.Sqrt, scale=100.0)
            ab = single.tile([128, N_BLK, D], BF16)
            for k in range(N_BLK):
                nc.scalar.mul(out=ab[:, k, :], in_=af[:, k, :],
                              mul=sca[:, k:k + 1])

            # ---- index table (tokens pre-wrapped host-side)
            idxs = single.tile([128, N_BLK, IDX_COLS], mybir.dt.int16)
            nc.sync.dma_start(out=idxs, in_=idx_in.ap())

            lossacc = single.tile([128, N_BLK], F32)

            # ---- Phase 1: gather + dots + softmax per 128-anchor block
            for k in range(N_BLK):
                cand = candp.tile([128, N_CAND, D], BF16)
                nc.gpsimd.dma_gather(
                    cand[:], mhat[:], idxs[:, k, :], N_IDX, N_IDX, D,
                    single_packet=False)
                prod = prodp.tile([128, N_CAND, D], BF16)
                nc.vector.tensor_tensor(
                    out=prod, in0=cand, in1=_bcast_mid(ab[:, k, :], N_CAND),
                    op=mybir.AluOpType.mult)
                # single-stage reduce: fp32-internal accumulation; 1-port DVE
                # mode (2-port modes would stall gpsimd SWDGE ring writes).
                logits = small.tile([128, N_CAND], F32)
                nc.vector.tensor_reduce(
                    out=logits, in_=prod, axis=mybir.AxisListType.X,
                    op=mybir.AluOpType.add)
                escr = small.tile([128, N_CAND], BF16)
                den = small.tile([128, 1], F32)
                # |logit| <= 10 so exp never overflows; skip max-subtraction.
                nc.scalar.activation(out=escr, in_=logits, func=AF.Exp,
                                     accum_out=den)
                lden = small.tile([128, 1], F32)
                nc.scalar.activation(out=lden, in_=den, func=AF.Ln)
                nc.vector.tensor_tensor(
                    out=lossacc[:, k:k + 1], in0=lden, in1=logits[:, 0:1],
                    op=mybir.AluOpType.subtract)

            nc.sync.dma_start(out=out.ap(), in_=lossacc)

    nc.compile()
    return nc


def make_in_maps(anchor_embedding, match_embedding, neg_idx):
    match = np.ascontiguousarray(np.asarray(match_embedding), dtype=np.float32)
    anchors = np.ascontiguousarray(np.asarray(anchor_embedding), dtype=np.float32)
    nidx = np.asarray(neg_idx).astype(np.int64)

    i = np.arange(N_IDX)
    b_l, n = i % 128, i // 128  # token i = n*128 + b -> dst[b, n, :]
    in_maps = []
    for c in range(N_CORES):
        lo = c * B_SHARD
        cand_idx = np.concatenate(
            [np.arange(lo, lo + B_SHARD, dtype=np.int64)[:, None],
             nidx[lo:lo + B_SHARD]], axis=1).astype(np.int16)  # [2048, 50]
        toks = np.empty((N_BLK, N_IDX), np.int16)
        for k in range(N_BLK):
            toks[k] = cand_idx[k * 128 + b_l, n]
        # dma_gather index wrap: token i read from [i % 16, i // 16],
        # replicated into each 16-partition group (one per gpsimd core).
        sb = toks.reshape(N_BLK, IDX_COLS, 16).transpose(2, 0, 1)  # [16,k,s]
        idx_host = np.tile(sb, (8, 1, 1))
        in_maps.append({
            "match": match,
            "anchors": anchors[lo:lo + B_SHARD],
            "idx": idx_host,
        })
    return in_maps


_NC_CACHE = None


def kernel(anchor_embedding, match_embedding, neg_idx) -> np.ndarray:
    global _NC_CACHE
    if _NC_CACHE is None:
        _NC_CACHE = build_bass()
    nc = _NC_CACHE
    in_maps = make_in_maps(anchor_embedding, match_embedding, neg_idx)
    res = run_bass_kernel_spmd(nc, in_maps, core_ids=list(range(N_CORES)))
    total = sum(float(r["out"].astype(np.float64).sum()) for r in res.results)
    return np.asarray(total / math.log(10.0), dtype=np.float32)



# revision 2
# speedup vs baseline: 1.0366x; 1.0366x over previous
"""ConflateLoss Trainium2 kernel.

loss = -sum_b log_softmax(10 * cos_sim(anchor_b, match[cand_idx_b]))[0] / ln(10)
with B=16384, D=128, 50 candidates per anchor (slot 0 = positive b, 1..49 = neg_idx).

Strategy (8 NeuronCores, data-parallel over B). The dominant per-call cost in
this environment is host->device input staging (~13 GB/s), so inputs are
shipped minimal: bf16 match SHARD (1/8 per core, all-gathered on device),
bf16 anchors shard, and a compact un-replicated int16 index table. Per core
that is ~1.2 MB instead of the 10.6 MB a replicated-f32 layout needs.

Device pipeline per core:
  - DRAM->DRAM stage + AllGather the 2048-row bf16 match shard -> full
    16384-row table in Shared DRAM.
  - Anchors: nsq via Square+reduce, scale 10/||a|| folded into bf16 anchors.
  - Index table [16, blk, 400] replicated to all 8 gpsimd groups via 8 DMAs.
  - Per 128-anchor block: dma_gather 6400 raw bf16 rows; dots = reduce(cand *
    ab); cand norms = reduce(Square(cand)); logits = dots * rsqrt(nsq);
    softmax partials via Exp(accum)->Ln.
  Output [128, 16] per-core partial losses; host sums and divides by ln(10).
"""

import math

import numpy as np
import ml_dtypes

import concourse.bacc as bacc
import concourse.bass as bass
import concourse.tile as tile
from concourse import mybir
from concourse.bass_utils import run_bass_kernel_spmd

B = 16384
D = 128
N_NEG = 49
N_CAND = 50
N_CORES = 8
B_SHARD = B // N_CORES  # 2048 anchors / match rows per core
N_BLK = B_SHARD // 128  # 16 blocks of 128 anchors
N_IDX = 128 * N_CAND  # 6400 gathered rows per block
IDX_COLS = N_IDX // 16  # 400 (dma_gather index wrap: token i at [i%16, i//16])

F32 = mybir.dt.float32
BF16 = mybir.dt.bfloat16
AF = mybir.ActivationFunctionType


def _bcast_mid(ap: bass.AP, count: int) -> bass.AP:
    """[128, D] AP -> [128, count, D] with a stride-0 middle dim."""
    assert len(ap.ap) == 2
    return bass.AP(
        tensor=ap.tensor,
        offset=ap.offset,
        ap=[list(ap.ap[0]), [0, count], list(ap.ap[1])],
    )


def build_bass() -> bacc.Bacc:
    nc = bacc.Bacc("TRN2", debug=False, num_devices=N_CORES)

    mshard_in = nc.dram_tensor("mshard", [B_SHARD, D], BF16, kind="ExternalInput")
    anchors_in = nc.dram_tensor("anchors", [B_SHARD, D], BF16, kind="ExternalInput")
    idx_in = nc.dram_tensor("idx", [16, N_BLK, IDX_COLS], mybir.dt.int16,
                            kind="ExternalInput")
    out = nc.dram_tensor("out", [128, N_BLK], F32, kind="ExternalOutput")

    with tile.TileContext(nc) as tc:
        with (
            tc.tile_pool(name="dram", bufs=1, space="DRAM") as dram_pool,
            tc.tile_pool(name="single", bufs=1) as single,
            tc.tile_pool(name="cand", bufs=3) as candp,
            tc.tile_pool(name="prod", bufs=2) as prodp,
            tc.tile_pool(name="csq", bufs=2) as csqp,
            tc.tile_pool(name="small", bufs=4) as small,
        ):
            # ---- match shard -> internal DRAM -> AllGather to full table
            mstage = dram_pool.tile([B_SHARD, D], BF16)
            nc.sync.dma_start(out=mstage, in_=mshard_in.ap())
            mfull = dram_pool.tile([B, D], BF16, addr_space="Shared")
            nc.gpsimd.collective_compute(
                kind="AllGather",
                op=mybir.AluOpType.bypass,
                replica_groups=[list(range(N_CORES))],
                ins=[mstage[:]],
                outs=[mfull[:]],
            )

            # ---- anchors: bf16, scaled by 10/||a|| (gamma folded)
            anch_r = anchors_in.ap().rearrange("(k p) d -> p k d", p=128)
            af = single.tile([128, N_BLK, D], BF16)
            nc.sync.dma_start(out=af, in_=anch_r)
            asq = single.tile([128, N_BLK, D], F32)
            nc.scalar.activation(out=asq, in_=af, func=AF.Square)
            nsqa = single.tile([128, N_BLK], F32)
            nc.vector.tensor_reduce(
                out=nsqa, in_=asq, axis=mybir.AxisListType.X,
                op=mybir.AluOpType.add)
            ra = single.tile([128, N_BLK], F32)
            nc.vector.reciprocal(out=ra, in_=nsqa)
            sca = single.tile([128, N_BLK], F32)
            # sqrt(100/nsq) = 10/||a||  (gamma=10 folded into the anchor)
            nc.scalar.activation(out=sca, in_=ra, func=AF.Sqrt, scale=100.0)
            ab = single.tile([128, N_BLK, D], BF16)
            for k in range(N_BLK):
                nc.scalar.mul(out=ab[:, k, :], in_=af[:, k, :],
                              mul=sca[:, k:k + 1])

            # ---- index table: replicate compact [16, ...] into all 8 groups
            idxs = single.tile([128, N_BLK, IDX_COLS], mybir.dt.int16)
            for g in range(8):
                nc.sync.dma_start(out=idxs[16 * g:16 * (g + 1)],
                                  in_=idx_in.ap())

            lossacc = single.tile([128, N_BLK], F32)

            # ---- per 128-anchor block: gather + dots + norms + softmax
            for k in range(N_BLK):
                cand = candp.tile([128, N_CAND, D], BF16)
                nc.gpsimd.dma_gather(
                    cand[:], mfull[:], idxs[:, k, :], N_IDX, N_IDX, D,
                    single_packet=False)
                prod = prodp.tile([128, N_CAND, D], BF16)
                nc.vector.tensor_tensor(
                    out=prod, in0=cand, in1=_bcast_mid(ab[:, k, :], N_CAND),
                    op=mybir.AluOpType.mult)
                dots = small.tile([128, N_CAND], F32)
                nc.vector.tensor_reduce(
                    out=dots, in_=prod, axis=mybir.AxisListType.X,
                    op=mybir.AluOpType.add)
                csq = csqp.tile([128, N_CAND, D], BF16)
                nc.scalar.activation(out=csq, in_=cand, func=AF.Square)
                nsq = small.tile([128, N_CAND], F32)
                nc.vector.tensor_reduce(
                    out=nsq, in_=csq, axis=mybir.AxisListType.X,
                    op=mybir.AluOpType.add)
                rn = small.tile([128, N_CAND], F32)
                nc.vector.reciprocal(out=rn, in_=nsq)
                rs = small.tile([128, N_CAND], F32)
                nc.scalar.activation(out=rs, in_=rn, func=AF.Sqrt)
                logits = small.tile([128, N_CAND], F32)
                nc.vector.tensor_tensor(
                    out=logits, in0=dots, in1=rs,
                    op=mybir.AluOpType.mult)
                escr = small.tile([128, N_CAND], BF16)
                den = small.tile([128, 1], F32)
                # |logit| <= 10 so exp never overflows; skip max-subtraction.
                nc.scalar.activation(out=escr, in_=logits, func=AF.Exp,
                                     accum_out=den)
                lden = small.tile([128, 1], F32)
                nc.scalar.activation(out=lden, in_=den, func=AF.Ln)
                nc.vector.tensor_tensor(
                    out=lossacc[:, k:k + 1], in0=lden, in1=logits[:, 0:1],
                    op=mybir.AluOpType.subtract)

            nc.sync.dma_start(out=out.ap(), in_=lossacc)

    nc.compile()
    return nc


def make_in_maps(anchor_embedding, match_embedding, neg_idx):
    match = np.asarray(match_embedding, dtype=np.float32).astype(
        ml_dtypes.bfloat16)
    anchors = np.asarray(anchor_embedding, dtype=np.float32).astype(
        ml_dtypes.bfloat16)
    nidx = np.asarray(neg_idx).astype(np.int64)

    i = np.arange(N_IDX)
    b_l, n = i % 128, i // 128  # token i = n*128 + b -> dst[b, n, :]
    in_maps = []
    for c in range(N_CORES):
        lo = c * B_SHARD
        cand_idx = np.concatenate(
            [np.arange(lo, lo + B_SHARD, dtype=np.int64)[:, None],
             nidx[lo:lo + B_SHARD]], axis=1).astype(np.int16)  # [2048, 50]
        toks = np.empty((N_BLK, N_IDX), np.int16)
        for k in range(N_BLK):
            toks[k] = cand_idx[k * 128 + b_l, n]
        # dma_gather index wrap: token i read from [i % 16, i // 16];
        # replication into the 8 gpsimd groups happens on-device.
        idx_host = np.ascontiguousarray(
            toks.reshape(N_BLK, IDX_COLS, 16).transpose(2, 0, 1))  # [16,k,s]
        in_maps.append({
            "mshard": np.ascontiguousarray(match[lo:lo + B_SHARD]),
            "anchors": np.ascontiguousarray(anchors[lo:lo + B_SHARD]),
            "idx": idx_host,
        })
    return in_maps


_NC_CACHE = None


def kernel(anchor_embedding, match_embedding, neg_idx) -> np.ndarray:
    global _NC_CACHE
    if _NC_CACHE is None:
        _NC_CACHE = build_bass()
    nc = _NC_CACHE
    in_maps = make_in_maps(anchor_embedding, match_embedding, neg_idx)
    res = run_bass_kernel_spmd(nc, in_maps, core_ids=list(range(N_CORES)))
    total = sum(float(r["out"].astype(np.float64).sum()) for r in res.results)
    return np.asarray(total / math.log(10.0), dtype=np.float32)


# revision 5
# speedup vs baseline: 1.0407x; 1.0039x over previous
"""ConflateLoss Trainium2 kernel.

loss = -sum_b log_softmax(10 * cos_sim(anchor_b, match[cand_idx_b]))[0] / ln(10)
with B=16384, D=128, 50 candidates per anchor (slot 0 = positive b, 1..49 = neg_idx).

Strategy (8 NeuronCores, data-parallel over B). The dominant per-call cost in
this environment is host->device input staging (~13 GB/s), so inputs are
shipped minimal: bf16 match SHARD (1/8 per core, all-gathered on device),
bf16 anchors shard, and a compact un-replicated int16 index table. Per core
that is ~1.2 MB instead of the 10.6 MB a replicated-f32 layout needs.

Device pipeline per core:
  - DRAM->DRAM stage + AllGather the 2048-row bf16 match shard -> full
    16384-row table in Shared DRAM.
  - Anchors: nsq via Square+reduce, scale 10/||a|| folded into bf16 anchors.
  - Index table [16, blk, 400] replicated to all 8 gpsimd groups via 8 DMAs.
  - Per 128-anchor block: dma_gather 6400 raw bf16 rows (alternating between
    2 SWDGE queues so consecutive gathers overlap); dots = reduce(cand * ab)
    and cand norm-squares = reduce(Square(cand)), both accumulated to bf16
    staging tiles (2x DVE mode; ~0.4% relative partials are far inside the
    tolerance since the loss sums 16k anchors).
  - Batched tail: logits = dots * rsqrt(nsq) for all 16 blocks at once, then
    softmax partials via Exp(accum)->Ln. ACT functions are grouped so the
    whole kernel needs only 2 activation-table loads (Square/Sqrt/Copy early,
    Exp/Ln in the tail) instead of ~2 per block.
  Output [128, 16] per-core partial losses; host sums and divides by ln(10).
"""
import contextlib

import math

import numpy as np
import ml_dtypes

import concourse.bacc as bacc
import concourse.bass as bass
import concourse.tile as tile
from concourse import mybir
from concourse.bass_utils import run_bass_kernel_spmd

B = 16384
D = 128
N_NEG = 49
N_CAND = 50
N_CORES = 8
B_SHARD = B // N_CORES  # 2048 anchors / match rows per core
N_BLK = B_SHARD // 128  # 16 blocks of 128 anchors
N_IDX = 128 * N_CAND  # 6400 gathered rows per block
IDX_COLS = N_IDX // 16  # 400 (dma_gather index wrap: token i at [i%16, i//16])

F32 = mybir.dt.float32
BF16 = mybir.dt.bfloat16
AF = mybir.ActivationFunctionType


def _bcast_mid(ap: bass.AP, count: int) -> bass.AP:
    """[128, D] AP -> [128, count, D] with a stride-0 middle dim."""
    assert len(ap.ap) == 2
    return bass.AP(
        tensor=ap.tensor,
        offset=ap.offset,
        ap=[list(ap.ap[0]), [0, count], list(ap.ap[1])],
    )


N_QUEUES = 2  # SWDGE queues; gathers alternate so two are in flight


def build_bass() -> bacc.Bacc:
    nc = bacc.Bacc("TRN2", debug=False, num_devices=N_CORES,
                   num_swdge_queues=N_QUEUES)

    mshard_in = nc.dram_tensor("mshard", [B_SHARD, D], BF16, kind="ExternalInput")
    anchors_in = nc.dram_tensor("anchors", [B_SHARD, D], BF16, kind="ExternalInput")
    idx_in = nc.dram_tensor("idx", [16, N_BLK, IDX_COLS], mybir.dt.int16,
                            kind="ExternalInput")
    out = nc.dram_tensor("out", [128, N_BLK], F32, kind="ExternalOutput")

    with tile.TileContext(nc) as tc:
        with (
            tc.tile_pool(name="dram", bufs=1, space="DRAM") as dram_pool,
            tc.tile_pool(name="single", bufs=1) as single,
            tc.tile_pool(name="cand", bufs=3) as candp,
            tc.tile_pool(name="prod", bufs=2) as prodp,
            tc.tile_pool(name="csq", bufs=2) as csqp,
            tc.tile_pool(name="small", bufs=4) as small,
        ):
            # ---- match shard -> internal DRAM -> AllGather to full table
            mstage = dram_pool.tile([B_SHARD, D], BF16)
            nc.sync.dma_start(out=mstage, in_=mshard_in.ap())
            mfull = dram_pool.tile([B, D], BF16, addr_space="Shared")
            nc.gpsimd.collective_compute(
                kind="AllGather",
                op=mybir.AluOpType.bypass,
                replica_groups=[list(range(N_CORES))],
                ins=[mstage[:]],
                outs=[mfull[:]],
            )

            # ---- anchors: bf16, scaled by 10/||a|| (gamma folded)
            anch_r = anchors_in.ap().rearrange("(k p) d -> p k d", p=128)
            af = single.tile([128, N_BLK, D], BF16)
            nc.sync.dma_start(out=af, in_=anch_r)
            asq = single.tile([128, N_BLK, D], F32)
            nc.scalar.activation(out=asq, in_=af, func=AF.Square)
            nsqa = single.tile([128, N_BLK], F32)
            nc.vector.tensor_reduce(
                out=nsqa, in_=asq, axis=mybir.AxisListType.X,
                op=mybir.AluOpType.add)
            ra = single.tile([128, N_BLK], F32)
            nc.vector.reciprocal(out=ra, in_=nsqa)
            sca = single.tile([128, N_BLK], F32)
            # sqrt(100/nsq) = 10/||a||  (gamma=10 folded into the anchor)
            nc.scalar.activation(out=sca, in_=ra, func=AF.Sqrt, scale=100.0)
            ab = single.tile([128, N_BLK, D], BF16)
            for k in range(N_BLK):
                nc.scalar.mul(out=ab[:, k, :], in_=af[:, k, :],
                              mul=sca[:, k:k + 1])

            # ---- index table: replicate compact [16, ...] into all 8 groups
            idxs = single.tile([128, N_BLK, IDX_COLS], mybir.dt.int16)
            for g in range(8):
                nc.sync.dma_start(out=idxs[16 * g:16 * (g + 1)],
                                  in_=idx_in.ap())

            dotsall = single.tile([128, N_BLK, N_CAND], BF16, tag="dotsall")
            nsqall = single.tile([128, N_BLK, N_CAND], BF16, tag="nsqall")

            # ---- per 128-anchor block: gather + dots + norm-squares
            with nc.allow_low_precision(
                    reason="bf16 partials are ~0.4% relative; the loss sums "
                           "16k anchors so the error washes out"):
                for k in range(N_BLK):
                    cand = candp.tile([128, N_CAND, D], BF16)
                    nc.gpsimd.dma_gather(
                        cand[:], mfull[:], idxs[:, k, :], N_IDX, N_IDX, D,
                        single_packet=False, queue_num=k % N_QUEUES)
                    prod = prodp.tile([128, N_CAND, D], BF16)
                    nc.vector.tensor_tensor(
                        out=prod, in0=cand,
                        in1=_bcast_mid(ab[:, k, :], N_CAND),
                        op=mybir.AluOpType.mult)
                    nc.vector.tensor_reduce(
                        out=dotsall[:, k, :], in_=prod,
                        axis=mybir.AxisListType.X,
                        op=mybir.AluOpType.add)
                    csq = csqp.tile([128, N_CAND, D], BF16)
                    nc.scalar.activation(out=csq, in_=cand, func=AF.Square)
                    nc.vector.tensor_reduce(
                        out=nsqall[:, k, :], in_=csq,
                        axis=mybir.AxisListType.X,
                        op=mybir.AluOpType.add)

            # ---- batched tail: logits = dots * rsqrt(nsq); softmax partials
            rcp = single.tile([128, N_BLK, N_CAND], F32, tag="rcp")
            nc.vector.reciprocal(out=rcp, in_=nsqall)
            rsq = single.tile([128, N_BLK, N_CAND], F32, tag="rsq")
            nc.scalar.activation(out=rsq, in_=rcp, func=AF.Sqrt)
            logits = single.tile([128, N_BLK, N_CAND], F32, tag="logits")
            nc.vector.tensor_tensor(out=logits, in0=dotsall, in1=rsq,
                                    op=mybir.AluOpType.mult)
            den = single.tile([128, N_BLK], F32, tag="den")
            for k in range(N_BLK):
                escr = small.tile([128, N_CAND], BF16)
                # |logit| <= 10 so exp never overflows; skip max-subtraction.
                nc.scalar.activation(out=escr, in_=logits[:, k, :],
                                     func=AF.Exp,
                                     accum_out=den[:, k:k + 1])
            lden = single.tile([128, N_BLK], F32, tag="lden")
            nc.scalar.activation(out=lden, in_=den, func=AF.Ln)
            lossacc = single.tile([128, N_BLK], F32)
            nc.vector.tensor_tensor(
                out=lossacc, in0=lden, in1=logits[:, :, 0],
                op=mybir.AluOpType.subtract)

            nc.sync.dma_start(out=out.ap(), in_=lossacc)

    nc.compile()
    return nc


def make_in_maps(anchor_embedding, match_embedding, neg_idx):
    match = np.asarray(match_embedding, dtype=np.float32).astype(
        ml_dtypes.bfloat16)
    anchors = np.asarray(anchor_embedding, dtype=np.float32).astype(
        ml_dtypes.bfloat16)
    nidx = np.asarray(neg_idx).astype(np.int64)

    i = np.arange(N_IDX)
    b_l, n = i % 128, i // 128  # token i = n*128 + b -> dst[b, n, :]
    in_maps = []
    for c in range(N_CORES):
        lo = c * B_SHARD
        cand_idx = np.concatenate(
            [np.arange(lo, lo + B_SHARD, dtype=np.int64)[:, None],
             nidx[lo:lo + B_SHARD]], axis=1).astype(np.int16)  # [2048, 50]
        toks = np.empty((N_BLK, N_IDX), np.int16)
        for k in range(N_BLK):
            toks[k] = cand_idx[k * 128 + b_l, n]
        # dma_gather index wrap: token i read from [i % 16, i // 16];
        # replication into the 8 gpsimd groups happens on-device.
        idx_host = np.ascontiguousarray(
            toks.reshape(N_BLK, IDX_COLS, 16).transpose(2, 0, 1))  # [16,k,s]
        in_maps.append({
            "mshard": np.ascontiguousarray(match[lo:lo + B_SHARD]),
            "anchors": np.ascontiguousarray(anchors[lo:lo + B_SHARD]),
            "idx": idx_host,
        })
    return in_maps


_NC_CACHE = None


def kernel(anchor_embedding, match_embedding, neg_idx) -> np.ndarray:
    global _NC_CACHE
    if _NC_CACHE is None:
        _NC_CACHE = build_bass()
    nc = _NC_CACHE
    in_maps = make_in_maps(anchor_embedding, match_embedding, neg_idx)
    res = run_bass_kernel_spmd(nc, in_maps, core_ids=list(range(N_CORES)))
    total = sum(float(r["out"].astype(np.float64).sum()) for r in res.results)
    return np.asarray(total / math.log(10.0), dtype=np.float32)


# revision 7
# speedup vs baseline: 1.9696x; 1.8926x over previous
"""ConflateLoss Trainium2 kernel.

loss = -sum_b log_softmax(10 * cos_sim(anchor_b, match[cand_idx_b]))[0] / ln(10)
with B=16384, D=128, 50 candidates per anchor (slot 0 = positive b, 1..49 = neg_idx).

Strategy (8 NeuronCores, data-parallel over B). The dominant per-call cost in
this environment is host->device input staging (~13 GB/s), so inputs are
shipped minimal: bf16 match SHARD (1/8 per core, all-gathered on device),
bf16 anchors shard, and a compact un-replicated int16 index table. Per core
that is ~1.2 MB instead of the 10.6 MB a replicated-f32 layout needs.

Device pipeline per core:
  - DRAM->DRAM stage + AllGather the 2048-row bf16 match shard -> full
    16384-row table in Shared DRAM.
  - Anchors: nsq via Square+reduce, scale 10/||a|| folded into bf16 anchors.
  - Index table [16, blk, 400] replicated to all 8 gpsimd groups via 8 DMAs.
  - Per 128-anchor block: dma_gather 6400 raw bf16 rows (alternating between
    2 SWDGE queues so consecutive gathers overlap); dots = reduce(cand * ab)
    and cand norm-squares = reduce(Square(cand)), both accumulated to bf16
    staging tiles (2x DVE mode; ~0.4% relative partials are far inside the
    tolerance since the loss sums 16k anchors).
  - Batched tail: logits = dots * rsqrt(nsq) for all 16 blocks at once, then
    softmax partials via Exp(accum)->Ln. ACT functions are grouped so the
    whole kernel needs only 2 activation-table loads (Square/Sqrt/Copy early,
    Exp/Ln in the tail) instead of ~2 per block.
  Output [128, 16] per-core partial losses; host sums and divides by ln(10).
"""
import contextlib

import math

import numpy as np
import ml_dtypes

import concourse.bacc as bacc
import concourse.bass as bass
import concourse.tile as tile
from concourse import mybir
from concourse.bass_utils import run_bass_kernel_spmd

B = 16384
D = 128
N_NEG = 49
N_CAND = 50
N_CORES = 8
B_SHARD = B // N_CORES  # 2048 anchors / match rows per core
N_BLK = B_SHARD // 128  # 16 blocks of 128 anchors
N_IDX = 128 * N_CAND  # 6400 gathered rows per block
IDX_COLS = N_IDX // 16  # 400 (dma_gather index wrap: token i at [i%16, i//16])

F32 = mybir.dt.float32
BF16 = mybir.dt.bfloat16
AF = mybir.ActivationFunctionType


def _bcast_mid(ap: bass.AP, count: int) -> bass.AP:
    """[128, D] AP -> [128, count, D] with a stride-0 middle dim."""
    assert len(ap.ap) == 2
    return bass.AP(
        tensor=ap.tensor,
        offset=ap.offset,
        ap=[list(ap.ap[0]), [0, count], list(ap.ap[1])],
    )


N_QUEUES = 2  # SWDGE queues; gathers alternate so two are in flight

# One merged input buffer per core: each extra ExternalInput costs ~0.5-1ms
# of per-call transfer overhead in this environment, so mshard + anchors +
# idx ship as a single bf16-typed blob (idx bytes bitcast back to int16).
MSHARD_ELEMS = B_SHARD * D
ANCH_ELEMS = B_SHARD * D
IDX_ELEMS = 16 * N_BLK * IDX_COLS
BLOB_ELEMS = MSHARD_ELEMS + ANCH_ELEMS + IDX_ELEMS


def build_bass() -> bacc.Bacc:
    nc = bacc.Bacc("TRN2", debug=False, num_devices=N_CORES,
                   num_swdge_queues=N_QUEUES)

    blob_in = nc.dram_tensor("blob", [BLOB_ELEMS], BF16, kind="ExternalInput")
    bap = blob_in.ap()
    mshard_ap = bap[0:MSHARD_ELEMS].rearrange("(r d) -> r d", d=D)
    anch_ap = bap[MSHARD_ELEMS:MSHARD_ELEMS + ANCH_ELEMS]
    idx_ap = bap[MSHARD_ELEMS + ANCH_ELEMS:BLOB_ELEMS].bitcast(
        mybir.dt.int16).rearrange("(a b c) -> a b c", b=N_BLK, c=IDX_COLS)
    out = nc.dram_tensor("out", [128, N_BLK], F32, kind="ExternalOutput")

    with tile.TileContext(nc) as tc:
        with (
            tc.tile_pool(name="dram", bufs=1, space="DRAM") as dram_pool,
            tc.tile_pool(name="single", bufs=1) as single,
            tc.tile_pool(name="cand", bufs=3) as candp,
            tc.tile_pool(name="prod", bufs=2) as prodp,
            tc.tile_pool(name="csq", bufs=2) as csqp,
            tc.tile_pool(name="small", bufs=4) as small,
        ):
            # ---- match shard -> internal DRAM -> AllGather to full table
            mstage = dram_pool.tile([B_SHARD, D], BF16)
            nc.sync.dma_start(out=mstage, in_=mshard_ap)
            mfull = dram_pool.tile([B, D], BF16, addr_space="Shared")
            nc.gpsimd.collective_compute(
                kind="AllGather",
                op=mybir.AluOpType.bypass,
                replica_groups=[list(range(N_CORES))],
                ins=[mstage[:]],
                outs=[mfull[:]],
            )

            # ---- anchors: bf16, scaled by 10/||a|| (gamma folded)
            anch_r = anch_ap.rearrange("(k p d) -> p k d", p=128, d=D)
            af = single.tile([128, N_BLK, D], BF16)
            nc.sync.dma_start(out=af, in_=anch_r)
            asq = single.tile([128, N_BLK, D], F32)
            nc.scalar.activation(out=asq, in_=af, func=AF.Square)
            nsqa = single.tile([128, N_BLK], F32)
            nc.vector.tensor_reduce(
                out=nsqa, in_=asq, axis=mybir.AxisListType.X,
                op=mybir.AluOpType.add)
            ra = single.tile([128, N_BLK], F32)
            nc.vector.reciprocal(out=ra, in_=nsqa)
            sca = single.tile([128, N_BLK], F32)
            # sqrt(100/nsq) = 10/||a||  (gamma=10 folded into the anchor)
            nc.scalar.activation(out=sca, in_=ra, func=AF.Sqrt, scale=100.0)
            ab = single.tile([128, N_BLK, D], BF16)
            for k in range(N_BLK):
                nc.scalar.mul(out=ab[:, k, :], in_=af[:, k, :],
                              mul=sca[:, k:k + 1])

            # ---- index table: replicate compact [16, ...] into all 8 groups
            idxs = single.tile([128, N_BLK, IDX_COLS], mybir.dt.int16)
            for g in range(8):
                nc.sync.dma_start(out=idxs[16 * g:16 * (g + 1)],
                                  in_=idx_ap)

            dotsall = single.tile([128, N_BLK, N_CAND], BF16, tag="dotsall")
            nsqall = single.tile([128, N_BLK, N_CAND], BF16, tag="nsqall")

            # ---- per 128-anchor block: gather + dots + norm-squares
            with nc.allow_low_precision(
                    reason="bf16 partials are ~0.4% relative; the loss sums "
                           "16k anchors so the error washes out"):
                for k in range(N_BLK):
                    cand = candp.tile([128, N_CAND, D], BF16)
                    nc.gpsimd.dma_gather(
                        cand[:], mfull[:], idxs[:, k, :], N_IDX, N_IDX, D,
                        single_packet=False, queue_num=k % N_QUEUES)
                    prod = prodp.tile([128, N_CAND, D], BF16)
                    nc.vector.tensor_tensor(
                        out=prod, in0=cand,
                        in1=_bcast_mid(ab[:, k, :], N_CAND),
                        op=mybir.AluOpType.mult)
                    nc.vector.tensor_reduce(
                        out=dotsall[:, k, :], in_=prod,
                        axis=mybir.AxisListType.X,
                        op=mybir.AluOpType.add)
                    csq = csqp.tile([128, N_CAND, D], BF16)
                    nc.scalar.activation(out=csq, in_=cand, func=AF.Square)
                    nc.vector.tensor_reduce(
                        out=nsqall[:, k, :], in_=csq,
                        axis=mybir.AxisListType.X,
                        op=mybir.AluOpType.add)

            # ---- batched tail: logits = dots * rsqrt(nsq); softmax partials
            rcp = single.tile([128, N_BLK, N_CAND], F32, tag="rcp")
            nc.vector.reciprocal(out=rcp, in_=nsqall)
            rsq = single.tile([128, N_BLK, N_CAND], F32, tag="rsq")
            nc.scalar.activation(out=rsq, in_=rcp, func=AF.Sqrt)
            logits = single.tile([128, N_BLK, N_CAND], F32, tag="logits")
            nc.vector.tensor_tensor(out=logits, in0=dotsall, in1=rsq,
                                    op=mybir.AluOpType.mult)
            den = single.tile([128, N_BLK], F32, tag="den")
            for k in range(N_BLK):
                escr = small.tile([128, N_CAND], BF16)
                # |logit| <= 10 so exp never overflows; skip max-subtraction.
                nc.scalar.activation(out=escr, in_=logits[:, k, :],
                                     func=AF.Exp,
                                     accum_out=den[:, k:k + 1])
            lden = single.tile([128, N_BLK], F32, tag="lden")
            nc.scalar.activation(out=lden, in_=den, func=AF.Ln)
            lossacc = single.tile([128, N_BLK], F32)
            nc.vector.tensor_tensor(
                out=lossacc, in0=lden, in1=logits[:, :, 0],
                op=mybir.AluOpType.subtract)

            nc.sync.dma_start(out=out.ap(), in_=lossacc)

    nc.compile()
    return nc


def make_in_maps(anchor_embedding, match_embedding, neg_idx):
    match = np.asarray(match_embedding, dtype=np.float32).astype(
        ml_dtypes.bfloat16)
    anchors = np.asarray(anchor_embedding, dtype=np.float32).astype(
        ml_dtypes.bfloat16)
    nidx = np.asarray(neg_idx).astype(np.int64)

    i = np.arange(N_IDX)
    b_l, n = i % 128, i // 128  # token i = n*128 + b -> dst[b, n, :]
    in_maps = []
    for c in range(N_CORES):
        lo = c * B_SHARD
        cand_idx = np.concatenate(
            [np.arange(lo, lo + B_SHARD, dtype=np.int64)[:, None],
             nidx[lo:lo + B_SHARD]], axis=1).astype(np.int16)  # [2048, 50]
        toks = np.empty((N_BLK, N_IDX), np.int16)
        for k in range(N_BLK):
            toks[k] = cand_idx[k * 128 + b_l, n]
        # dma_gather index wrap: token i read from [i % 16, i // 16];
        # replication into the 8 gpsimd groups happens on-device.
        idx_host = np.ascontiguousarray(
            toks.reshape(N_BLK, IDX_COLS, 16).transpose(2, 0, 1))  # [16,k,s]
        blob = np.concatenate([
            np.ascontiguousarray(match[lo:lo + B_SHARD]).ravel().view(np.int16),
            np.ascontiguousarray(anchors[lo:lo + B_SHARD]).ravel().view(np.int16),
            idx_host.ravel(),
        ]).view(ml_dtypes.bfloat16)
        in_maps.append({"blob": blob})
    return in_maps


_NC_CACHE = None


def kernel(anchor_embedding, match_embedding, neg_idx) -> np.ndarray:
    global _NC_CACHE
    if _NC_CACHE is None:
        _NC_CACHE = build_bass()
    nc = _NC_CACHE
    in_maps = make_in_maps(anchor_embedding, match_embedding, neg_idx)
    res = run_bass_kernel_spmd(nc, in_maps, core_ids=list(range(N_CORES)))
    total = sum(float(r["out"].astype(np.float64).sum()) for r in res.results)
    return np.asarray(total / math.log(10.0), dtype=np.float32)
